# revision 45
# baseline (speedup 1.0000x reference)
"""GNN message passing (u_mul_e -> segment_sum) on 8 Trainium2 NeuronCores.

out[v] = sum_{e=(u->v)} h[u] * w[e]

Strategy (edge/graph parallelism, dst-strip sharded -> no collectives):
  - Host snake-deals the 782 global 128-dst strips across the 8 cores by
    descending edge count (DEAL: matched cell-size distributions minimize
    the cross-core-max padding), buckets each core's edges by
    (strip, src chunk), sorts edges by src within each cell (SRCSORT: HBM
    row locality for the gather), and pads each cell to a multiple of 128
    edges uniformly across cores (same instruction stream = SPMD).
  - Device, per group of strips:
      * dma_gather pulls h rows (bf16, 256B) for each edge's src (one
        descriptor per edge; src chunked into <=25000-row windows so
        indices fit int16). 256B elems are the HW sweet spot: 64B-elem
        descriptors measured ~2x slower end to end.
      * DVE builds the one-hot P[p=e, j, t] = (offs_e == j) in a j-major
        layout (JMAJOR): the offs broadcast has stride-1 inner dim so it
        runs as a 4x-mode DVE copy (vs the 1x Activation broadcast), then
        one big 2x is_equal against a host-materialized j-major iota, and
        one mult applies w to the gathered rows. Per-tile fused
        tensor_scalar one-hots measured ~7x slower on HW (per-instruction
        overhead), so everything stays batched per group.
      * PE computes P^T @ msg per strip (lhsT = strided j-major slice),
        accumulating the [128, 32] segment sums in PSUM over the strip's
        tiles.
  - Per-strip results are copied to an SBUF output buffer, DMA'd out once,
    and the host inverts the strip deal when assembling the full output.
"""

import os
import sys

sys.path.insert(0, "/opt/trn_rl_repo")

import numpy as np
import ml_dtypes

BF16 = ml_dtypes.bfloat16

# Full-problem configuration (hardcoded; kernel.py must be self-contained).
# Every knob has a baked-in default and a K_<name> env override for probing.
FULL_CFG = dict(
    N=100000,   # nodes
    E=1600000,  # edges
    D=32,       # feature dim
    NC=8,       # cores
    CH=4,       # src chunks (int16 path only; unused when INDIRECT)
    SPG=7,      # dst strips (128 nodes each) per pipeline group
    INDIRECT=0,  # (broken in this runtime) indirect DMA gather w/ int32 idx
    EW=128,      # gathered elems per edge: 32 (64B, slower on HW!) | 128 (256B)
    SRCSORT=1,   # sort edges by src within each cell (HBM locality)
    DEAL=1,      # snake-deal global dst strips across cores (balances padding)
    FUSED=0,     # per-tile fused one-hot is ~7x slower on HW than batched ops
    JMAJOR=1,    # one-hot in [p, j, t] layout: broadcast via DVE 4x copy
                 # (inner dim stride-1) instead of the 1x Activation broadcast
    SUBG=0,      # tiles per sub-gather call (0 = one call per group/chunk)
    SWDGEQ=1,    # SWDGE queues (int16 dma_gather path only, max 4)
    REPEAT=1,    # pipeline repetitions inside the NEFF (timing probes)
    ONLY_GATHER=0,  # probe: emit only the gather side
    SKIP_GATHER=0,  # probe: emit only the compute side
)


def _derive(cfg):
    c = dict(cfg)
    for k in ("SPG", "INDIRECT", "EW", "SRCSORT", "DEAL", "FUSED", "JMAJOR",
              "SUBG", "SWDGEQ", "REPEAT", "ONLY_GATHER", "SKIP_GATHER", "CH"):
        c.setdefault(k, FULL_CFG[k])
        v = os.environ.get("K_" + k)
        if v is not None:
            c[k] = int(v)
    c["INDIRECT"] = bool(c["INDIRECT"])
    c["SRCSORT"] = bool(c["SRCSORT"])
    c["FUSED"] = bool(c["FUSED"])
    c["JMAJOR"] = bool(c["JMAJOR"]) and not c["FUSED"]
    c["ONLY_GATHER"] = bool(c["ONLY_GATHER"])
    c["SKIP_GATHER"] = bool(c["SKIP_GATHER"])
    if c["INDIRECT"]:
        c["CH"] = 1  # int32 indices: no src chunking needed
    assert c["N"] % c["NC"] == 0
    c["NPC"] = c["N"] // c["NC"]               # dst nodes per core
    c["S"] = -(-c["NPC"] // 128)               # strips per core
    assert c["N"] % c["CH"] == 0
    c["CHN"] = c["N"] // c["CH"]               # h rows per src chunk
    assert c["INDIRECT"] or c["CHN"] <= 32767
    c["G"] = -(-c["S"] // c["SPG"])            # strip groups
    return c


def _plan(src, dst, w, cfg):
    """Bucket/sort/pad edges; build per-core device input streams."""
    c = cfg
    N, E, NC, NPC, S, CH, CHN = c["N"], c["E"], c["NC"], c["NPC"], c["S"], c["CH"], c["CHN"]

    src = np.asarray(src).astype(np.int64).ravel()
    dst = np.asarray(dst).astype(np.int64).ravel()
    w = np.asarray(w, dtype=np.float32).ravel()

    if c["DEAL"]:
        # Global 128-dst strips, snake-dealt to cores by descending edge
        # count so every core sees a near-identical multiset of cell sizes
        # (NB is a cross-core max; matched distributions minimize padding).
        GS = -(-N // 128)
        gstrip = dst >> 7
        offs = dst & 127
        scnt = np.bincount(gstrip, minlength=GS)
        order_s = np.argsort(-scnt, kind="stable")
        deal_core = np.empty(GS, dtype=np.int64)
        deal_slot = np.empty(GS, dtype=np.int64)
        j = np.arange(GS)
        rnd, pos = j // NC, j % NC
        pos = np.where(rnd % 2 == 0, pos, NC - 1 - pos)
        deal_core[order_s] = pos
        deal_slot[order_s] = rnd
        assert GS <= S * NC
        core = deal_core[gstrip]
        strip = deal_slot[gstrip]
        slot_map = np.full((NC, S), -1, dtype=np.int64)
        slot_map[deal_core, deal_slot] = np.arange(GS)
    else:
        core = dst // NPC
        rem = dst - core * NPC
        strip = rem >> 7
        offs = rem & 127
        slot_map = None
    chunk = src // CHN
    if c["INDIRECT"]:
        lsrc = src.astype(np.int32)
    else:
        lsrc = (src - chunk * CHN).astype(np.int16)

    cellkey = (core * S + strip) * CH + chunk
    counts = np.bincount(cellkey, minlength=NC * S * CH)
    NB = -(-counts.reshape(NC, S, CH).max(axis=0) // 128)  # [S, CH] blocks per cell

    # Tile order: (group, chunk, strip-in-group, block).
    cell_tile_start = np.zeros((S, CH), dtype=np.int64)
    t_acc = 0
    for g in range(c["G"]):
        s0, s1 = g * c["SPG"], min((g + 1) * c["SPG"], S)
        for ch in range(CH):
            for s in range(s0, s1):
                cell_tile_start[s, ch] = t_acc
                t_acc += NB[s, ch]
    T = int(t_acc)
    assert T == int(NB.sum())
    TE = T * 128

    # Scatter each edge to its position in its core's padded stream.
    # SRCSORT additionally orders edges by src within each cell so the
    # gather's HBM reads have row-buffer locality (order within a cell is
    # free: all its edges share strip+chunk, so any permutation is valid).
    if c["SRCSORT"]:
        order = np.lexsort((src, chunk, strip, core))
    else:
        order = np.lexsort((chunk, strip, core))
    core_s = core[order]
    starts = np.zeros(NC * S * CH + 1, dtype=np.int64)
    np.cumsum(counts, out=starts[1:])
    rank = np.arange(E, dtype=np.int64) - starts[cellkey[order]]
    pos = cell_tile_start[strip[order], chunk[order]] * 128 + rank

    pad_idx = -1 if int(os.environ.get("K_NEGPAD", "0")) else 0
    idx_dt = np.int32 if c["INDIRECT"] else np.int16
    idx_stream = np.full((NC, TE), pad_idx, dtype=idx_dt)
    offs_stream = np.full((NC, TE), -1.0, dtype=np.float32)
    w_stream = np.zeros((NC, TE), dtype=np.float32)
    idx_stream[core_s, pos] = lsrc[order]
    offs_stream[core_s, pos] = offs[order]
    w_stream[core_s, pos] = w[order]

    if c["INDIRECT"]:
        # idx: [NC, 128, T] partition-major per tile (edge (t, p) -> [p, t]),
        # matching the indirect DMA's pairing of offset ap with out ap.
        idx_wrapped = np.ascontiguousarray(
            idx_stream.reshape(NC, T, 128).transpose(0, 2, 1)
        )
    else:
        # idx: wrapped per (group, chunk) run: within-run element i ->
        # [i%16, i//16], replicated across the 8 GPSIMD core groups
        # (128 partitions total).
        idx_wrapped = np.zeros((NC, 16, TE // 16), dtype=np.int16)
        run_t = 0
        for g in range(c["G"]):
            s0, s1 = g * c["SPG"], min((g + 1) * c["SPG"], S)
            for ch in range(CH):
                n = int(NB[s0:s1, ch].sum())
                if n == 0:
                    continue
                seg = idx_stream[:, run_t * 128:(run_t + n) * 128]
                idx_wrapped[:, :, run_t * 8:(run_t + n) * 8] = (
                    seg.reshape(NC, -1, 16).transpose(0, 2, 1)
                )
                run_t += n
        assert run_t == T

    # offs/w: wrapped globally per 128-edge tile: element i -> [i%128, i//128].
    offs_arr = np.ascontiguousarray(offs_stream.reshape(NC, T, 128).transpose(0, 2, 1))
    w_arr = np.ascontiguousarray(w_stream.reshape(NC, T, 128).transpose(0, 2, 1))

    return NB, idx_wrapped, offs_arr, w_arr, slot_map


def _dma_gather_raw(gp, mybir, out_ap, in_ap, idxs_ap, num_idxs, elem_size,
                    elem_step, queue_num=0, single_packet=False):
    """dma_gather with elem_size below 256B (the bass wrapper's 256B assert
    is a transpose-path restriction; the HW descriptor only requires the
    source row stride to be a multiple of 256B, which elem_step provides).
    Verified bit-exact on HW for elem_size=32 bf16 (64B)."""
    assert in_ap.ap[-1][1] == out_ap.ap[-1][1] == elem_size
    assert in_ap.ap[0][0] == elem_step
    stride_bytes = elem_step * mybir.dt.size(in_ap.dtype)
    stride_bytes_256 = stride_bytes // 256
    assert stride_bytes % 256 == 0 and 0 < stride_bytes_256 < 256
    _in_ap = gp.lower_ap_dma(in_ap, for_custom_bir_dma=True)
    return gp.add_instruction(
        mybir.InstDMAGatherAnt(
            name=gp.bass.get_next_instruction_name(),
            ins=[*_in_ap, gp.lower_ap(idxs_ap),
                 gp.lower_val_access(gp.to_reg(num_idxs))],
            outs=[gp.lower_ap(out_ap)],
            transpose=False,
            num_idxs=num_idxs,
            elem_size=elem_size,
            stride_bytes_256=stride_bytes_256,
            gen_mode=0,
            single_packet=single_packet,
            queue_num=queue_num,
            sbuf_tokens_per_rank=0,
            sbuf_free_dim_per_rank=0,
            sbuf_free_dim_pad_per_rank=0,
            sbuf_byte_offset=0,
        )
    )


def _group_counts(NB, c):
    """Per-group per-chunk tile counts and the max group size."""
    S, CH, G, SPG = c["S"], c["CH"], c["G"], c["SPG"]
    g_ncg = []
    for g in range(G):
        s0, s1 = g * SPG, min((g + 1) * SPG, S)
        g_ncg.append([int(NB[s0:s1, ch].sum()) for ch in range(CH)])
    return g_ncg, max(sum(x) for x in g_ncg)


def _build(NB, cfg):
    """Build the Bass program (shared by all 8 cores)."""
    from concourse import bacc, tile, mybir

    c = cfg
    N, S, CH, CHN, G, SPG = c["N"], c["S"], c["CH"], c["CHN"], c["G"], c["SPG"]
    dt = mybir.dt
    T = int(NB.sum())

    g_ncg, NTG_MAX = _group_counts(NB, c)

    fused = c["FUSED"]
    repeat = c["REPEAT"]
    nq = c["SWDGEQ"]
    sdt = dt.bfloat16 if c["JMAJOR"] else dt.float32  # offs/wt stream dtype
    iota_cols = 128 * NTG_MAX if c["JMAJOR"] else 128

    nc = bacc.Bacc(None, num_swdge_queues=nq)
    if c["INDIRECT"]:
        h_ext = nc.declare_dram_parameter("h", [N, 32], dt.bfloat16, isOutput=False)
        idx_ext = nc.declare_dram_parameter("idx", [128, T], dt.int32, isOutput=False)
    else:
        h_ext = nc.declare_dram_parameter("h", [N, 128], dt.bfloat16, isOutput=False)
        idx_ext = nc.declare_dram_parameter("idx", [128, T * 8], dt.int16, isOutput=False)
    offs_ext = nc.declare_dram_parameter("offs", [128, T], sdt, isOutput=False)
    wt_ext = nc.declare_dram_parameter("wt", [128, T], sdt, isOutput=False)
    iota_ext = nc.declare_dram_parameter("iota", [128, iota_cols], dt.bfloat16, isOutput=False)
    out_ext = nc.declare_dram_parameter("out", [S * 128, 32], dt.float32, isOutput=True)

    with tile.TileContext(nc) as tc:
        with (
            tc.tile_pool(name="const", bufs=1) as cpool,
            tc.tile_pool(name="gp", bufs=2) as gpool,
            tc.tile_pool(name="pwp", bufs=8 if fused else 2) as pwpool,
            tc.tile_pool(name="sm", bufs=2) as smpool,
            tc.tile_pool(name="outp", bufs=1) as opool,
            tc.tile_pool(name="ps", bufs=4, space="PSUM") as pspool,
        ):
            iota_t = cpool.tile([128, iota_cols], dt.bfloat16)
            nc.sync.dma_start(out=iota_t[:], in_=iota_ext[:])
            out_sbuf = opool.tile([128, S * 32], dt.float32)
            if c["ONLY_GATHER"]:
                nc.vector.memset(out_sbuf[:], 0.0)

            for _rep in range(repeat):
                _emit_pipeline(
                    nc, tile, mybir, dt, NB, c, g_ncg, NTG_MAX, fused,
                    iota_t, out_sbuf,
                    gpool, pwpool, smpool, pspool,
                    h_ext, idx_ext, offs_ext, wt_ext,
                )

            nc.sync.dma_start(
                out=out_ext[:].rearrange("(s p) d -> p s d", p=128),
                in_=out_sbuf[:].rearrange("p (s d) -> p s d", d=32),
            )
    nc.finalize()
    return nc


def _emit_pipeline(
    nc, tile, mybir, dt, NB, c, g_ncg, NTG_MAX, fused,
    iota_t, out_sbuf,
    gpool, pwpool, smpool, pspool,
    h_ext, idx_ext, offs_ext, wt_ext,
):
    S, CH, CHN, G, SPG = c["S"], c["CH"], c["CHN"], c["G"], c["SPG"]
    toff = 0
    for g in range(G):
        s0, s1 = g * SPG, min((g + 1) * SPG, S)
        ncg = g_ncg[g]
        ntg = sum(ncg)
        if ntg == 0:
            for s in range(s0, s1):
                nc.vector.memset(out_sbuf[:, s * 32:(s + 1) * 32], 0.0)
            continue

        indirect = c["INDIRECT"]
        # gathered elem width (bf16 elems)
        ew = 32 if indirect else c["EW"]
        subg = c["SUBG"]  # tiles per sub-gather call (0=off)
        nqs = max(1, c["SWDGEQ"])
        only_gather = c["ONLY_GATHER"]
        skip_gather = c["SKIP_GATHER"]

        gbuf = gpool.tile([128, NTG_MAX * ew], dt.bfloat16, tag="gbuf")
        if skip_gather:
            # probe mode: zero gbuf (never gather-written) so reads are legal
            nc.vector.memset(gbuf[:], 0.0)
        if indirect:
            idx_t = smpool.tile([128, NTG_MAX], dt.int32, tag="idx")
            nc.sync.dma_start(
                out=idx_t[:, :ntg], in_=idx_ext[:, toff:toff + ntg]
            )
        else:
            idx_t = smpool.tile([128, NTG_MAX * 8], dt.int16, tag="idx")
            nc.sync.dma_start(
                out=idx_t[:, : ntg * 8], in_=idx_ext[:, toff * 8:(toff + ntg) * 8]
            )
        sdt = dt.bfloat16 if c["JMAJOR"] else dt.float32
        offs_t = smpool.tile([128, NTG_MAX], sdt, tag="offs")
        wt_t = smpool.tile([128, NTG_MAX], sdt, tag="wt")

        nc.sync.dma_start(out=offs_t[:, :ntg], in_=offs_ext[:, toff:toff + ntg])
        nc.sync.dma_start(out=wt_t[:, :ntg], in_=wt_ext[:, toff:toff + ntg])

        g3 = gbuf[:].rearrange("p (t e) -> p t e", e=ew)
        if indirect and not skip_gather:
            from concourse.bass import IndirectOffsetOnAxis

            step = subg if subg else ntg
            for o in range(0, ntg, step):
                m = min(step, ntg - o)
                nc.gpsimd.indirect_dma_start(
                    out=g3[:, o:o + m, :],
                    out_offset=None,
                    in_=h_ext[:],
                    in_offset=IndirectOffsetOnAxis(
                        ap=idx_t[:, o:o + m], axis=0
                    ),
                )
        elif not skip_gather:
            co = 0
            for ch in range(CH):
                n = ncg[ch]
                if n == 0:
                    continue
                step = subg if subg else n
                for o in range(0, n, step):
                    m = min(step, n - o)
                    if ew < 128:
                        _dma_gather_raw(
                            nc.gpsimd, mybir,
                            out_ap=g3[:, co + o:co + o + m, :],
                            in_ap=h_ext[ch * CHN:(ch + 1) * CHN, :ew],
                            idxs_ap=idx_t[:, (co + o) * 8:(co + o + m) * 8],
                            num_idxs=m * 128,
                            elem_size=ew,
                            elem_step=128,
                            single_packet=(m * 128 <= 1024) if subg else False,
                            queue_num=ch % nqs,
                        )
                    else:
                        nc.gpsimd.dma_gather(
                            out_ap=g3[:, co + o:co + o + m, :],
                            in_ap=h_ext[ch * CHN:(ch + 1) * CHN, :ew],
                            idxs_ap=idx_t[:, (co + o) * 8:(co + o + m) * 8],
                            num_idxs=m * 128,
                            num_idxs_reg=m * 128,
                            elem_size=ew,
                            elem_step=128,
                            # single-packet desc-gen faults above 1024 idxs
                            single_packet=(m * 128 <= 1024) if subg else False,
                            queue_num=ch % nqs,
                        )
                co += n

        if only_gather:
            toff += ntg
            continue

        jmajor = c["JMAJOR"] and not fused
        if not fused:
            pw = pwpool.tile([128, NTG_MAX * 128], dt.bfloat16, tag="pw")
            if jmajor:
                # [p, j, t] layout: inner dim (t) is stride-1 in every AP, so
                # the broadcast runs as a DVE 4x copy and is_equal as 2x TT.
                pwj = pw[:].rearrange("p (j t) -> p j t", t=NTG_MAX)
                iotaj = iota_t[:].rearrange("p (j t) -> p j t", t=NTG_MAX)
                nc.vector.tensor_copy(
                    out=pwj[:, :, :ntg],
                    in_=offs_t[:, :ntg].unsqueeze(1).broadcast_to(
                        [128, 128, ntg]),
                )
                nc.vector.tensor_tensor(
                    out=pwj[:, :, :ntg],
                    in0=pwj[:, :, :ntg],
                    in1=iotaj[:, :, :ntg],
                    op=mybir.AluOpType.is_equal,
                )
            else:
                pw3 = pw[:].rearrange("p (t e) -> p t e", e=128)
                # Broadcast per-edge dst offsets across the 128 one-hot cols.
                nc.scalar.activation(
                    out=pw3[:, :ntg, :],
                    in_=offs_t[:, :ntg].unsqueeze(2).broadcast_to(
                        [128, ntg, 128]),
                    func=mybir.ActivationFunctionType.Copy,
                )
                # One-hot: P[e, j] = (offs_e == j)
                nc.vector.tensor_tensor(
                    out=pw3[:, :ntg, :],
                    in0=iota_t[:].unsqueeze(1).broadcast_to([128, ntg, 128]),
                    in1=pw3[:, :ntg, :],
                    op=mybir.AluOpType.is_equal,
                )
            # msg = h[src] * w (in place on the used 32 columns)
            nc.vector.tensor_tensor(
                out=g3[:, :ntg, 0:32],
                in0=g3[:, :ntg, 0:32],
                in1=wt_t[:, :ntg].unsqueeze(2).broadcast_to([128, ntg, 32]),
                op=mybir.AluOpType.mult,
            )

        chunk_base = np.concatenate([[0], np.cumsum(ncg)]).astype(int)
        for s in range(s0, s1):
            nb = int(NB[s].sum())
            if nb == 0:
                nc.vector.memset(out_sbuf[:, s * 32:(s + 1) * 32], 0.0)
                continue
            ps = pspool.tile([128, 32], dt.float32)
            bi = 0
            for ch in range(CH):
                nbs = int(NB[s, ch])
                if nbs == 0:
                    continue
                lt0 = int(chunk_base[ch] + NB[s0:s, ch].sum())
                for b in range(nbs):
                    t = lt0 + b
                    if fused:
                        # P_w[e, j] = (offs_e == j) * w_e in one DVE op
                        pwb = pwpool.tile([128, 128], dt.bfloat16, tag="pwb")
                        nc.vector.tensor_scalar(
                            out=pwb[:],
                            in0=iota_t[:],
                            scalar1=offs_t[:, t:t + 1],
                            scalar2=wt_t[:, t:t + 1],
                            op0=mybir.AluOpType.is_equal,
                            op1=mybir.AluOpType.mult,
                        )
                        lhs = pwb[:]
                    else:
                        lhs = pwj[:, :, t] if jmajor else pw[:, t * 128:(t + 1) * 128]
                    nc.tensor.matmul(
                        out=ps[:],
                        lhsT=lhs,
                        rhs=g3[:, t, 0:32],
                        start=(bi == 0),
                        stop=(bi == nb - 1),
                    )
                    bi += 1
            nc.scalar.copy(out=out_sbuf[:, s * 32:(s + 1) * 32], in_=ps[:])
        toff += ntg


def _make_in_maps(h, idx_wrapped, offs_arr, w_arr, c, NB):
    N, D, NC = c["N"], c["D"], c["NC"]
    if c["INDIRECT"]:
        h_pad = np.ascontiguousarray(
            np.asarray(h, dtype=np.float32).astype(BF16)
        )
        idx_of = lambda i: idx_wrapped[i]
    else:
        h_pad = np.zeros((N, 128), dtype=BF16)
        h_pad[:, :D] = np.asarray(h, dtype=np.float32).astype(BF16)
        idx_of = lambda i: np.ascontiguousarray(np.tile(idx_wrapped[i], (8, 1)))
    if c["JMAJOR"] and not c["FUSED"]:
        _, ntg_max = _group_counts(NB, c)
        # iota in [p, (j t)] layout: value j repeated ntg_max times
        iota = np.broadcast_to(
            np.repeat(np.arange(128, dtype=np.float32), ntg_max).astype(BF16),
            (128, 128 * ntg_max),
        ).copy()
        offs_arr = offs_arr.astype(BF16)
        w_arr = w_arr.astype(BF16)
    else:
        iota = np.broadcast_to(
            np.arange(128, dtype=np.float32).astype(BF16), (128, 128)
        ).copy()
    return [
        {
            "h": h_pad,
            "idx": idx_of(i),
            "offs": offs_arr[i],
            "wt": w_arr[i],
            "iota": iota,
        }
        for i in range(NC)
    ]


def _assemble_out(core_outs, c, slot_map):
    """core_outs[i]: [S*128, D] per core -> full [N, D]."""
    N, D, NC, NPC, S = c["N"], c["D"], c["NC"], c["NPC"], c["S"]
    out = np.empty((N, D), dtype=np.float32)
    if slot_map is None:
        for i in range(NC):
            out[i * NPC:(i + 1) * NPC] = core_outs[i][:NPC]
    else:
        for i in range(NC):
            a = core_outs[i]
            for s in range(S):
                gs = int(slot_map[i, s])
                if gs < 0:
                    continue
                r0 = gs * 128
                n = min(128, N - r0)
                out[r0:r0 + n] = a[s * 128:s * 128 + n]
    return out


def run_cfg(h, w, src, dst, cfg, trace=False):
    from concourse.bass_utils import run_bass_kernel_spmd

    c = _derive(cfg)
    NC = c["NC"]

    NB, idx_wrapped, offs_arr, w_arr, slot_map = _plan(src, dst, w, c)
    nc = _build(NB, c)

    in_maps = _make_in_maps(h, idx_wrapped, offs_arr, w_arr, c, NB)
    res = run_bass_kernel_spmd(nc, in_maps, list(range(NC)), trace=trace)
    out = _assemble_out([res.results[i]["out"] for i in range(NC)], c, slot_map)
    return out, res


def make_runner(h, w, src, dst, cfg):
    """Build a reusable jitted SPMD callable for timing: returns
    (run_once, assemble) where run_once() returns unblocked device arrays."""
    import jax
    import jax.numpy as jnp
    from jax.sharding import Mesh, PartitionSpec, NamedSharding
    from jax.experimental.shard_map import shard_map
    from concourse import bass2jax, mybir

    c = _derive(cfg)
    N, D, NC, NPC = c["N"], c["D"], c["NC"], c["NPC"]

    NB, idx_wrapped, offs_arr, w_arr, slot_map = _plan(src, dst, w, c)
    nc = _build(NB, c)

    in_maps = _make_in_maps(h, idx_wrapped, offs_arr, w_arr, c, NB)

    bass2jax.install_neuronx_cc_hook()
    partition_name = nc.partition_id_tensor.name if nc.partition_id_tensor else None
    in_names, out_names, out_avals, zero_shapes = [], [], [], []
    for alloc in nc.m.functions[0].allocations:
        if not isinstance(alloc, mybir.MemoryLocationSet):
            continue
        name = alloc.memorylocations[0].name
        if alloc.kind == "ExternalInput":
            if name != partition_name:
                in_names.append(name)
        elif alloc.kind == "ExternalOutput":
            out_names.append(name)
            shape = tuple(alloc.tensor_shape)
            dtype = mybir.dt.np(alloc.dtype)
            out_avals.append(jax.core.ShapedArray(shape, dtype))
            zero_shapes.append((shape, dtype))
    n_params = len(in_names)
    n_outs = len(out_avals)
    all_in_names = list(in_names) + list(out_names)
    if partition_name is not None:
        all_in_names.append(partition_name)

    def _body(*args):
        operands = list(args)
        if partition_name is not None:
            operands.append(bass2jax.partition_id_tensor())
        outs = bass2jax._bass_exec_p.bind(
            *operands,
            out_avals=tuple(out_avals),
            in_names=tuple(all_in_names),
            out_names=tuple(out_names),
            lowering_input_output_aliases=(),
            sim_require_finite=True,
            sim_require_nnan=True,
            nc=nc,
        )
        return tuple(outs)

    devices = jax.devices()[:NC]
    mesh = Mesh(np.asarray(devices), ("core",))
    donate = tuple(range(n_params, n_params + n_outs))
    sharded = jax.jit(
        shard_map(
            _body,
            mesh=mesh,
            in_specs=(PartitionSpec("core"),) * (n_params + n_outs),
            out_specs=(PartitionSpec("core"),) * n_outs,
            check_rep=False,
        ),
        donate_argnums=donate,
        keep_unused=True,
    )

    concat_in = [
        np.concatenate([np.asarray(in_maps[k][nm]) for k in range(NC)], axis=0)
        for nm in in_names
    ]
    shard = NamedSharding(mesh, PartitionSpec("core"))
    dev_in = [jax.device_put(a, shard) for a in concat_in]

    zeros_fn = jax.jit(
        lambda: tuple(
            jnp.zeros((NC * s[0], *s[1:]), dt) for (s, dt) in zero_shapes
        ),
        out_shardings=(shard,) * n_outs,
    )

    def run_once():
        zs = zeros_fn()
        return sharded(*dev_in, *zs)

    def assemble(out_arrs):
        o = np.asarray(out_arrs[0]).reshape(NC, -1, D)
        return _assemble_out([o[i] for i in range(NC)], c, slot_map)

    # chained executor: K back-to-back executions in ONE dispatch, each
    # feeding its output as the next call's out-operand (defeats CSE).
    def make_chain(k):
        def _chain_body(*args):
            ins, outs = args[:n_params], list(args[n_params:])
            for _ in range(k):
                outs = list(_body(*ins, *outs))
            return tuple(outs)

        return jax.jit(
            shard_map(
                _chain_body,
                mesh=mesh,
                in_specs=(PartitionSpec("core"),) * (n_params + n_outs),
                out_specs=(PartitionSpec("core"),) * n_outs,
                check_rep=False,
            ),
            donate_argnums=donate,
            keep_unused=True,
        )

    def run_chain(chain_fn):
        zs = zeros_fn()
        return chain_fn(*dev_in, *zs)

    return run_once, assemble, make_chain, run_chain


def kernel(**inputs):
    out, _ = run_cfg(
        inputs["h"], inputs["w"], inputs["src"], inputs["dst"], FULL_CFG
    )
    return out

